# revision 8
# baseline (speedup 1.0000x reference)
"""Multi-head attention (B=2, S=4096, E=512, H=8) on 8 Trainium2 NeuronCores.

Sharding v3: core c handles batch b = c//4 and head-pair hp = c%4 (heads
2*hp, 2*hp+1). Each core projects K/V/Q only for its two heads (128 of the
512 E dims) over the full batch, runs attention for all 4096 queries of
those heads, and emits the partial output projection attn_pair @ Wo[pair
rows]. The host sums the four partials per batch and adds bo. This removes
the 4x-redundant K/V projections of the batch-sharded layout.

Engine budget per core (measured/modelled):
- Tensor: QK streams 262k cols (109us) + AV 2048 stationary loads+65-col
  streams (~110us) + projections (~30us) -> ~250us busy.
- ACT: exp over 33.5M scores. Pure-ACT would be ~280us, so a fraction of
  key-blocks (DVE_KB) is evaluated on the Vector engine with a one-pass
  Schraudolph fast-exp (tensor_scalar -> int16 bits, bitcast to fp16),
  calibrated to ~1.5% weight error at full share; at 8/32 share the final
  output error stays ~5e-3 against the 2e-2 gate.
- AV accumulates in PSUM across all 32 key blocks (start=True on the first
  sub-tile zeroes the whole 2KB region; the other q-blocks ride on the
  pending-zero bytes), so the Vector engine does no numerator accumulation.
- Transposes of the normalized numerators run on the DMA engine (xbar
  transpose), not the PE.
"""

from contextlib import ExitStack

import numpy as np

import concourse.bass as bass
import concourse.tile as tile
from concourse import bacc, mybir
from concourse.bass_utils import run_bass_kernel_spmd

F32 = mybir.dt.float32
F16 = mybir.dt.float16
I16 = mybir.dt.int16
AF = mybir.ActivationFunctionType
ALU = mybir.AluOpType

B = 2
S = 4096  # keys = queries per batch
E = 512
H = 8
DH = 64
P = 128
NE = E // P  # 4 contraction chunks
KB = S // P  # 32 key blocks
NCH = 8  # query chunks
QCH = S // NCH  # 512 queries per chunk
QB = QCH // P  # 4 query blocks per chunk
SCALE = 1.0 / np.sqrt(DH)

# Schraudolph fast-exp on DVE: bits = s_raw*EXPA + EXPB, viewed as fp16.
EXPA = float(SCALE * 1024.0 * np.log2(np.e))
EXPB = float(15 * 1024 - 400)
# key blocks (mod 8) whose exp runs on the Vector engine instead of ACT
DVE_KB = (3, 7)


def emit(ctx: ExitStack, tc: tile.TileContext, io: dict):
    nc = tc.nc
    xT = io["xT"]  # [E, S] f16
    y = io["y"]  # [S, E] f32 partial out

    const = ctx.enter_context(tc.tile_pool(name="const", bufs=1))
    big = ctx.enter_context(tc.tile_pool(name="big", bufs=1))
    atp = ctx.enter_context(tc.tile_pool(name="atp", bufs=4))
    ntp = ctx.enter_context(tc.tile_pool(name="ntp", bufs=4))
    attnp = ctx.enter_context(tc.tile_pool(name="attnp", bufs=2))
    yp = ctx.enter_context(tc.tile_pool(name="yp", bufs=2))
    rp = ctx.enter_context(tc.tile_pool(name="rp", bufs=4))
    ps_sc = ctx.enter_context(tc.tile_pool(name="ps_sc", bufs=2, space="PSUM"))
    ps_acc = ctx.enter_context(tc.tile_pool(name="ps_acc", bufs=2, space="PSUM"))
    ps_y = ctx.enter_context(tc.tile_pool(name="ps_y", bufs=2, space="PSUM"))

    # --- weights / biases ------------------------------------------------
    w16 = {}
    for wn in ("Wq", "Wk", "Wv"):
        wt = const.tile([P, NE, P], F16, tag=f"w_{wn}", name=f"w_{wn}")
        for ec in range(NE):
            eng = nc.sync if ec % 2 == 0 else nc.gpsimd
            eng.dma_start(wt[:, ec, :], io[wn][ec * P : (ec + 1) * P, :])
        w16[wn] = wt
    wo_t = const.tile([P, E], F16, tag="wo")
    nc.sync.dma_start(wo_t[:], io["Wo"])

    bq_t = const.tile([P, 1], F32, tag="bq")
    bk_t = const.tile([P, 1], F32, tag="bk")
    nc.gpsimd.dma_start(bq_t[:], io["bq"])
    nc.gpsimd.dma_start(bk_t[:], io["bk"])
    # bv broadcast to [P, 128] (the pair's dims), viewed as [P, 2, 64]
    bv_b = const.tile([P, P], F32, tag="bv")
    src = io["bv"]
    nc.gpsimd.dma_start(
        bv_b[:], bass.AP(tensor=src.tensor, offset=src.offset, ap=[[0, P]] + list(src.ap))
    )

    # --- big SBUF tensors ------------------------------------------------
    xT16 = big.tile([P, NE, S], F16, tag="xT")
    kT16 = big.tile([P, S], F16, tag="kT")
    qTp = big.tile([P, 2, S], F16, tag="qTp")
    v16 = big.tile([P, KB, 2, DH + 1], F16, tag="v")
    # zero-pad qTp (head h occupies partition rows h*64..h*64+64)
    nc.vector.memset(qTp[DH:P, 0, :], 0.0)
    nc.vector.memset(qTp[0:DH, 1, :], 0.0)
    nc.vector.memset(v16[:, :, :, DH : DH + 1], 1.0)

    # --- x loads + projections, interleaved by S-quarter ------------------
    def load_x_quarter(qtr):
        c0 = qtr * 1024
        for ec in range(NE):
            eng = nc.sync if ec % 2 == 0 else nc.gpsimd
            eng.dma_start(
                xT16[:, ec, c0 : c0 + 1024], xT[ec * P : (ec + 1) * P, c0 : c0 + 1024]
            )

    def proj_quarter(qtr):
        for h512 in range(2):
            cols = slice(qtr * 1024 + h512 * 512, qtr * 1024 + h512 * 512 + 512)
            pk = ps_sc.tile([P, 2, 512], F32, tag="sc", name=f"pk{qtr}_{h512}")
            for ec in range(NE):
                nc.tensor.matmul(
                    pk[:, 0, :],
                    lhsT=w16["Wk"][:, ec, :],
                    rhs=xT16[:, ec, cols],
                    start=(ec == 0),
                    stop=(ec == NE - 1),
                )
            nc.vector.tensor_scalar_add(kT16[:, cols], pk[:, 0, :], bk_t[:])
            pq = ps_sc.tile([P, 2, 512], F32, tag="sc", name=f"pq{qtr}_{h512}")
            for ec in range(NE):
                nc.tensor.matmul(
                    pq[:, 0, :],
                    lhsT=w16["Wq"][:, ec, :],
                    rhs=xT16[:, ec, cols],
                    start=(ec == 0),
                    stop=(ec == NE - 1),
                )
            nc.vector.tensor_scalar_add(
                qTp[0:DH, 0, cols], pq[0:DH, 0, :], bq_t[0:DH]
            )
            nc.vector.tensor_scalar_add(
                qTp[DH:P, 1, cols], pq[DH:P, 0, :], bq_t[DH:P]
            )
        for sb8 in range(8):
            sb = qtr * 8 + sb8
            pv = ps_sc.tile([P, 2, 512], F32, tag="sc", name=f"pv{sb}")
            for ec in range(NE):
                nc.tensor.matmul(
                    pv[:, 0, 0:P],
                    lhsT=xT16[:, ec, sb * P : (sb + 1) * P],
                    rhs=w16["Wv"][:, ec, :],
                    start=(ec == 0),
                    stop=(ec == NE - 1),
                )
            nc.vector.tensor_add(
                v16[:, sb, :, 0:DH],
                pv[:, 0, 0:P].rearrange("p (h d) -> p h d", h=2),
                bv_b[:].rearrange("p (h d) -> p h d", h=2),
            )

    for qtr in range(4):
        load_x_quarter(qtr)
        proj_quarter(qtr)

    # --- attention: q-chunk outer, all key blocks inner -------------------
    def qk_pair(ch, kb):
        qcols = slice(ch * QCH, (ch + 1) * QCH)
        sct = ps_sc.tile([P, 2, 512], F32, tag="sc", name=f"sc{ch}_{kb}")
        for h in range(2):
            nc.tensor.matmul(
                sct[:, h, :],
                lhsT=kT16[:, kb * P : (kb + 1) * P],
                rhs=qTp[:, h, qcols],
                start=True,
                stop=True,
            )
        att = atp.tile([P, 2, 512], F16, tag="at", name=f"at{ch}_{kb}")
        if kb % 8 in DVE_KB:
            nc.vector.tensor_scalar(
                att[:].bitcast(I16), sct[:], EXPA, EXPB, op0=ALU.mult, op1=ALU.add
            )
        else:
            nc.scalar.activation(att[:], sct[:], AF.Exp, scale=float(SCALE))
        return att

    def av(accs, kb, att):
        for h in range(2):
            for qb in range(QB):
                nc.tensor.matmul(
                    accs[h][:, qb, :],
                    lhsT=att[:, h, qb * P : (qb + 1) * P],
                    rhs=v16[:, kb, h, :],
                    start=False,
                    stop=(kb == KB - 1),
                    skip_group_check=True,
                )

    def normalize(ch, accs):
        """Chunk-end: nt[q, dh-pair] = num/den in fp16; frees the acc tiles."""
        nts = []
        for qb in range(QB):
            nt = ntp.tile([P, P], F16, tag="nt", name=f"nt{ch}_{qb}")
            for h in range(2):
                rt = rp.tile([P, 1], F32, tag="r")
                nc.vector.reciprocal(rt[:], accs[h][:, qb, DH : DH + 1])
                nc.vector.tensor_scalar_mul(
                    nt[:, h * DH : (h + 1) * DH], accs[h][:, qb, 0:DH], rt[:]
                )
            nts.append(nt)
        return nts

    def finish_qb(ch, nt, qb):
        attnT = attnp.tile([P, P], F16, tag="attnT", name=f"aT{ch}_{qb}")
        nc.sync.dma_start_transpose(attnT[:], nt[:])
        py = ps_y.tile([P, E], F32, tag="y", name=f"py{ch}_{qb}")
        nc.tensor.matmul(py[:], lhsT=attnT[:], rhs=wo_t[:], start=True, stop=True)
        ysb = yp.tile([P, E], F32, tag="ysb")
        nc.vector.tensor_copy(ysb[:], py[:])
        qrow = ch * QCH + qb * P
        nc.gpsimd.dma_start(y[qrow : qrow + P, :], ysb[:])

    pending = []  # software pipeline: delay AV two key blocks behind QK
    deferred = []  # finish tail of the previous chunk, spread over this one
    for ch in range(NCH):
        accs = [
            ps_acc.tile([P, QB, DH + 1], F32, tag="acc", name=f"acc{ch}_{h}")
            for h in range(2)
        ]
        for h in range(2):
            nc.vector.memset(accs[h][:], 0.0)
        for kb in range(KB):
            att = qk_pair(ch, kb)
            pending.append((accs, kb, att))
            if len(pending) > 2:
                av(*pending.pop(0))
            if deferred and kb % 4 == 2:
                deferred.pop(0)()
        while pending:
            av(*pending.pop(0))
        nts = normalize(ch, accs)
        deferred = [
            (lambda c=ch, n=nt, q=qb: finish_qb(c, n, q))
            for qb, nt in enumerate(nts)
        ]
    for fn in deferred:
        fn()


def build():
    nc = bacc.Bacc("TRN2", target_bir_lowering=False, debug=False)
    io = {}
    io["xT"] = nc.dram_tensor("xT", [E, S], F16, kind="ExternalInput").ap()
    for n in ("Wq", "Wk", "Wv"):
        io[n] = nc.dram_tensor(n, [E, P], F16, kind="ExternalInput").ap()
    io["Wo"] = nc.dram_tensor("Wo", [P, E], F16, kind="ExternalInput").ap()
    for n in ("bq", "bk", "bv"):
        io[n] = nc.dram_tensor(n, [P], F32, kind="ExternalInput").ap()
    io["y"] = nc.dram_tensor("y", [S, E], F32, kind="ExternalOutput").ap()
    with tile.TileContext(nc) as tc:
        with ExitStack() as ctx:
            emit(ctx, tc, io)
    nc.compile()
    return nc


_NC = None


def _get_nc():
    global _NC
    if _NC is None:
        _NC = build()
    return _NC


def shard_inputs(x, Wq, bq, Wk, bk, Wv, bv, Wo, bo):
    x16 = np.asarray(x, dtype=np.float16)
    xTb = [np.ascontiguousarray(x16[b].T) for b in range(B)]
    Wq16 = np.asarray(Wq, dtype=np.float16)
    Wk16 = np.asarray(Wk, dtype=np.float16)
    Wv16 = np.asarray(Wv, dtype=np.float16)
    Wo16 = np.asarray(Wo, dtype=np.float16)
    maps = []
    for c in range(8):
        b, hp = c // 4, c % 4
        sl = slice(hp * P, (hp + 1) * P)
        maps.append(
            {
                "xT": xTb[b],
                "Wq": np.ascontiguousarray(Wq16[:, sl]),
                "Wk": np.ascontiguousarray(Wk16[:, sl]),
                "Wv": np.ascontiguousarray(Wv16[:, sl]),
                "Wo": np.ascontiguousarray(Wo16[sl, :]),
                "bq": np.ascontiguousarray(np.asarray(bq, np.float32)[sl]),
                "bk": np.ascontiguousarray(np.asarray(bk, np.float32)[sl]),
                "bv": np.ascontiguousarray(np.asarray(bv, np.float32)[sl]),
            }
        )
    return maps


def gather_output(results, bo):
    out = np.empty((B, S, E), dtype=np.float32)
    for b in range(B):
        acc = results[4 * b]["y"].astype(np.float32, copy=True)
        for hp in range(1, 4):
            acc += results[4 * b + hp]["y"]
        out[b] = acc + np.asarray(bo, np.float32)
    return out


def kernel(x, Wq, bq, Wk, bk, Wv, bv, Wo, bo):
    nc = _get_nc()
    maps = shard_inputs(x, Wq, bq, Wk, bk, Wv, bv, Wo, bo)
    res = run_bass_kernel_spmd(nc, maps, list(range(8)))
    return gather_output(res.results, bo)
